# revision 1
# baseline (speedup 1.0000x reference)
"""Causal self-attention (B=4, T=2048, C=1024, H=16) on 8 NeuronCores.

Sharding: core = (batch b, head-group g): data-parallel over B=4, tensor-
parallel over heads (2 groups x 8 heads).  Each core computes QKV + attention
for its 8 heads and the matching half of the c_proj contraction; the host
sums the two partial c_proj outputs per batch and adds b_proj.

Device layout notes:
  - all matmul operands bf16 (PE runs fp32 at 1/4 rate), PSUM f32
  - x, weights are pre-transposed on the host so every matmul contraction
    sits on the partition dim; no on-device transposes anywhere
  - QKV biases enter as K=1 rank-1 matmuls against a ones row
  - S is computed transposed ([keys, queries]); exp(S/8) on ScalarE with no
    max-subtraction (logits bounded ~+-4 for this problem's scale)
  - causality at tile granularity: k-tiles above the diagonal are skipped,
    diagonal tiles multiplied by precomputed 0/1 masks after exp; diagonal
    tiles further restrict S/exp/PV to their valid column range
  - softmax denominator = ones column appended to each head's V; PV matmul
    emits [y.T | denom] per (head, q-chunk); normalization = batched DVE
    reciprocal + one-hot-selector broadcast matmul + elementwise multiply

Scheduling notes (the performance-critical part):
  - the attention inner loop is ScalarE(exp)-bound; the PE would idle ~35%
    and the HAM activity monitor then clock-throttles it to 1.2 GHz.  To
    keep the PE dense, later head-pairs' QKV projection matmuls (and the
    second half of V) are drip-fed one-per-iteration into the attention
    k-loop as "filler" work, and each head's normalization is deferred into
    the next head's filler stream (its reciprocal would otherwise stall the
    in-order PE queue >3.4us and re-trip the throttle)
  - S+exp run DEPTH=3 k-tiles ahead of the PV consumer (software pipeline)
  - emission order == dependency-tracking order, so ensure() force-emits a
    pair's projection fillers before that pair's attention reads them
"""

import os

import numpy as np
import ml_dtypes

B, T, C, H = 4, 2048, 1024, 16
D = 64          # head dim
HL = 8          # heads per core
CL = HL * D     # 512 local channels
TQ = 512        # query chunk (matmul moving dim)
TK = 128        # key tile (psum partition dim)
NQC = T // TQ   # 4 query chunks
NKT = T // TK   # 16 key tiles
VW = HL * (D + 1)  # 520: V with per-head ones column

_prog = None
last_results = None  # BassKernelResults of the most recent run (for test.py)


def _build_program():
    import concourse.mybir as mybir
    import concourse.tile as tile
    from concourse import bacc

    f32 = mybir.dt.float32
    bf16 = mybir.dt.bfloat16
    EXP = mybir.ActivationFunctionType.Exp

    nc = bacc.Bacc("TRN2", target_bir_lowering=False, debug=False)

    xt_d = nc.dram_tensor("xt", [8, 128, T], bf16, kind="ExternalInput")
    wqk_d = nc.dram_tensor("wqk", [8, 128, 2 * CL], bf16, kind="ExternalInput")
    wv_d = nc.dram_tensor("wv", [8, 128, VW], bf16, kind="ExternalInput")
    bqk_d = nc.dram_tensor("bqk", [1, 2 * CL], bf16, kind="ExternalInput")
    bv_d = nc.dram_tensor("bv", [1, VW], bf16, kind="ExternalInput")
    wp_d = nc.dram_tensor("wp", [4, 128, C], bf16, kind="ExternalInput")
    mask_d = nc.dram_tensor("mask", [4, 128, TQ], bf16, kind="ExternalInput")
    out_d = nc.dram_tensor("out", [T, C], f32, kind="ExternalOutput")

    with tile.TileContext(nc) as tc:
        with (
            tc.tile_pool(name="persist", bufs=1) as pp,
            tc.tile_pool(name="ptpool", bufs=8) as ptp,
            tc.tile_pool(name="stage", bufs=3) as sp,
            tc.tile_pool(name="small", bufs=3) as smp,
            tc.tile_pool(name="psA", bufs=4, space="PSUM") as psA,
            tc.tile_pool(name="psF", bufs=2, space="PSUM") as psF,
            tc.tile_pool(name="psY", bufs=1, space="PSUM") as psY,
            tc.tile_pool(name="psB", bufs=1, space="PSUM") as psB,
        ):
            # ---- load everything ----
            xt = [pp.tile([128, T], bf16, name=f"xt{k}") for k in range(8)]
            wqk = [pp.tile([128, 2 * CL], bf16, name=f"wqk{k}") for k in range(8)]
            wv = [pp.tile([128, VW], bf16, name=f"wv{k}") for k in range(8)]
            wp = [pp.tile([128, C], bf16, name=f"wp{k}") for k in range(4)]
            maskt = [pp.tile([128, TQ], bf16, name=f"mask{j}") for j in range(4)]
            bqk_row = pp.tile([1, 2 * CL], bf16, name="bqk_row")
            bv_row = pp.tile([1, VW], bf16, name="bv_row")
            ones_row = pp.tile([1, TQ], bf16, name="ones_row")
            # one-hot selector matrices: bcast of row 32*s of a [97,512] tile
            # into 64 partitions via a rank-1 matmul (SBUF APs may only start
            # at partition 0/32/64/96, hence the 32-pitch); f32-allocated,
            # bitcast to f32r at the matmul for 4x PE throughput
            sel = [pp.tile([97, 64], f32, name=f"sel{i}") for i in range(4)]

            for k in range(8):
                nc.sync.dma_start(out=wqk[k][:], in_=wqk_d[k])
            for k in range(8):
                nc.sync.dma_start(out=xt[k][:], in_=xt_d[k])
            for k in range(8):
                nc.sync.dma_start(out=wv[k][:], in_=wv_d[k])
            for k in range(4):
                nc.sync.dma_start(out=wp[k][:], in_=wp_d[k])
            for j in range(4):
                nc.sync.dma_start(out=maskt[j][:], in_=mask_d[j])
            nc.sync.dma_start(out=bqk_row[:], in_=bqk_d[:])
            nc.sync.dma_start(out=bv_row[:], in_=bv_d[:])
            nc.vector.memset(ones_row[:], 1.0)
            for i in range(4):
                nc.vector.memset(sel[i][:], 0.0)
                nc.vector.memset(sel[i][32 * i : 32 * i + 1, :], 1.0)

            # ---- QKV projection ----
            # QT/KT in [channel, t] layout; channel tile g = head pair g
            qt = [pp.tile([128, T], bf16, name=f"qt{i}") for i in range(4)]
            kt = [pp.tile([128, T], bf16, name=f"kt{i}") for i in range(4)]
            # V in natural [t, channel] layout with a ones column per head
            vsb = [pp.tile([128, VW], bf16, name=f"v{i}") for i in range(NKT)]
            yt = [pp.tile([128, T], bf16, name=f"yt{i}") for i in range(4)]

            def emit_qk_steps(g):
                """One head-pair's Q.T and K.T projection as a list of
                single-matmul closures (PE filler units)."""
                steps = []
                for dst, off in ((qt, 0), (kt, CL)):
                    for j in range(NQC):
                        ph = {}

                        def step(k, ph=ph, dst=dst, off=off, j=j, g=g):
                            if k == 0:
                                ph["ps"] = psF.tile(
                                    [128, TQ], f32, name="ps_f", tag="fill"
                                )
                            if k < 8:
                                nc.tensor.matmul(
                                    ph["ps"][:],
                                    lhsT=wqk[k][:, off + g * 128 : off + (g + 1) * 128],
                                    rhs=xt[k][:, j * TQ : (j + 1) * TQ],
                                    start=(k == 0),
                                    stop=False,
                                )
                            else:
                                nc.tensor.matmul(
                                    ph["ps"][:],
                                    lhsT=bqk_row[0:1, off + g * 128 : off + (g + 1) * 128],
                                    rhs=ones_row[0:1, :],
                                    start=False,
                                    stop=True,
                                )
                                nc.vector.tensor_copy(
                                    out=dst[g][:, j * TQ : (j + 1) * TQ], in_=ph["ps"]
                                )

                        for k in range(9):
                            steps.append(lambda k=k, step=step: step(k))
                return steps

            def emit_v_steps(h2):
                """V projection for 4 heads as single-matmul closures."""
                w0 = h2 * (VW // 2)
                steps = []
                for it in range(NKT):
                    ph = {}

                    def step(k, ph=ph, it=it, w0=w0):
                        if k == 0:
                            ph["ps"] = psF.tile([128, TQ], f32, name="ps_v",
                                                tag="fill")
                        if k < 8:
                            nc.tensor.matmul(
                                ph["ps"][:, : VW // 2],
                                lhsT=xt[k][:, it * 128 : (it + 1) * 128],
                                rhs=wv[k][:, w0 : w0 + VW // 2],
                                start=(k == 0),
                                stop=False,
                            )
                        else:
                            nc.tensor.matmul(
                                ph["ps"][:, : VW // 2],
                                lhsT=ones_row[0:1, 0:128],
                                rhs=bv_row[0:1, w0 : w0 + VW // 2],
                                start=False,
                                stop=True,
                            )
                            nc.vector.tensor_copy(
                                out=vsb[it][:, w0 : w0 + VW // 2],
                                in_=ph["ps"][:, : VW // 2],
                            )

                    for k in range(9):
                        steps.append(lambda k=k, step=step: step(k))
                return steps

            # pre-phase (dense PE work, warms HAM): pair 0 QK + first V half
            # (heads 0-3); the second half drains via the filler stream well
            # before attn pair 2 (heads 4-7) consumes it
            for step in emit_qk_steps(0):
                step()
            for step in emit_v_steps(0):
                step()

            # ---- attention ----
            # the attention inner loop is ScalarE(exp)-bound; drip-feed the
            # NEXT pair's QKV matmuls into the PE queue as filler so the PE
            # stays dense (otherwise HAM clock-throttles it to 1.2 GHz)
            fillers = []  # list of (tag, closure)

            def drain(n):
                for _ in range(n):
                    if fillers:
                        fillers.pop(0)[1]()

            def ensure(tag):
                """Force-emit every queued step up to the last one of `tag`
                (emission order == dependency-tracking order, so a pair's
                projection steps MUST be emitted before its attention reads)."""
                while any(t == tag for t, _ in fillers):
                    fillers.pop(0)[1]()

            for h in range(HL):
                g2, po = h // 2, 64 * (h % 2)
                if h % 2 == 0 and g2 < 3:
                    fillers.extend((f"qk{g2 + 1}", s) for s in emit_qk_steps(g2 + 1))
                    if g2 == 0:
                        fillers.extend(("vh1", s) for s in emit_v_steps(1))
                if h % 2 == 0:
                    ensure(f"qk{g2}")
                    if g2 >= 2:
                        ensure("vh1")
                den_g = smp.tile([97, TQ], f32, name="den_g", tag="deng", bufs=4)
                nc.vector.memset(den_g[:], 1.0)  # rows between dens unused
                slots = []
                for qc in range(NQC):
                    ktop = (qc + 1) * (TQ // TK)  # causal: k tiles 0..ktop-1
                    yps = psY.tile([D + 1, TQ], f32, name="yps", tag="y")

                    # software pipeline: S+exp runs DEPTH tiles ahead of the
                    # PV consumer so the PE never waits on the exp
                    DEPTH = 3
                    pts = {}

                    def s_stage(ktl, qc=qc, g2=g2, po=po, pts=pts):
                        j = ktl - qc * (TQ // TK)
                        # diagonal tiles have no valid columns before col0
                        col0 = j * TK if j >= 0 else 0
                        ps_s = psA.tile([128, TQ], f32, name="ps_s", tag="mm512")
                        nc.tensor.matmul(
                            ps_s[:, col0:],
                            lhsT=kt[g2][po : po + 64, ktl * TK : (ktl + 1) * TK],
                            rhs=qt[g2][po : po + 64, qc * TQ + col0 : (qc + 1) * TQ],
                            start=True,
                            stop=True,
                        )
                        pt_t = ptp.tile([128, TQ], bf16, name="pt")
                        # P.T = exp(S.T/sqrt(D)); logits bounded, no max pass
                        nc.scalar.activation(
                            pt_t[:, col0:], ps_s[:, col0:], EXP, scale=0.125
                        )
                        if j >= 0:  # diagonal: zero the acausal corner
                            # on GPSIMD (idle): keeps the mask off the busy
                            # in-order DVE queue so PV never waits behind
                            # ysb/den/filler copies
                            nc.gpsimd.tensor_mul(
                                pt_t[:, col0:], pt_t[:, col0:], maskt[j][:, col0:]
                            )
                        pts[ktl] = (pt_t, col0)

                    def pv_stage(ktl, qc=qc, h=h, pts=pts, yps=yps, ktop=ktop):
                        pt_t, col0 = pts.pop(ktl)
                        nc.tensor.matmul(
                            yps[:, col0:],
                            lhsT=vsb[ktl][:, h * 65 : (h + 1) * 65],
                            rhs=pt_t[:, col0:],
                            start=(ktl == 0),
                            stop=(ktl == ktop - 1),
                        )

                    for ktl in range(ktop):
                        s_stage(ktl)
                        drain(2 if len(fillers) >= 80 else 1)
                        if ktl >= DEPTH:
                            pv_stage(ktl - DEPTH)
                    for ktl in range(max(0, ktop - DEPTH), ktop):
                        pv_stage(ktl)
                    # rows 0..63 = unnormalized y.T, row 64 = denominator
                    ysb = smp.tile([D + 1, TQ], f32, name="ysb", tag="ysb", bufs=14)
                    nc.vector.tensor_copy(out=ysb[:], in_=yps[:])
                    nc.vector.tensor_copy(
                        out=den_g[32 * qc : 32 * qc + 1, :], in_=ysb[64:65, :]
                    )
                    slots.append((h, qc, ysb))
                # batched 1/den for the head (DVE; free-size bound so one
                # [97,512] reciprocal costs the same as [1,512]).  The whole
                # normalization is pushed into the NEXT head's filler stream:
                # emitted inline it stalls the in-order PE queue ~4us at every
                # head boundary (reciprocal latency), re-tripping the HAM
                # throttle.  Reciprocal is split in half to avoid a DVE lump
                # that would delay the next head's mask-multiplies.
                rec_g = smp.tile([97, TQ], f32, name="rec_g", tag="recg", bufs=3)

                def norm_steps(rec_g=rec_g, den_g=den_g, slots=tuple(slots)):
                    steps = [
                        lambda: nc.vector.reciprocal(
                            rec_g[:, : TQ // 2], den_g[:, : TQ // 2]
                        ),
                        lambda: nc.vector.reciprocal(
                            rec_g[:, TQ // 2 :], den_g[:, TQ // 2 :]
                        ),
                    ]

                    def one(h_, qc, ysb):
                        g2_, po_ = h_ // 2, 64 * (h_ % 2)
                        bc = psB.tile([64, TQ], f32, name="bc", tag="b")
                        nc.tensor.matmul(
                            bc[:],
                            lhsT=sel[qc][:],
                            rhs=rec_g[:],
                            start=True,
                            stop=True,
                        )
                        nc.vector.tensor_mul(
                            yt[g2_][po_ : po_ + 64, qc * TQ : (qc + 1) * TQ],
                            ysb[0:64, :],
                            bc[:],
                        )

                    for h_, qc, ysb in slots:
                        steps.append(lambda h_=h_, qc=qc, ysb=ysb: one(h_, qc, ysb))
                    return steps

                # a few slots of lead so the reciprocal completes before its
                # broadcast matmuls reach the PE, but early enough that ysb
                # tiles are recycled within ~one head
                for i_, st in enumerate(norm_steps()):
                    fillers.insert(min(8 + i_, len(fillers)), ("norm", st))
            drain(len(fillers))

            # ---- c_proj (local half of the contraction) ----
            for it in range(NKT):
                for oc in range(2):
                    pso = psA.tile([128, TQ], f32, name="ps_o", tag="mm512")
                    for ic in range(4):
                        nc.tensor.matmul(
                            pso[:],
                            lhsT=yt[ic][:, it * 128 : (it + 1) * 128],
                            rhs=wp[ic][:, oc * TQ : (oc + 1) * TQ],
                            start=(ic == 0),
                            stop=(ic == 3),
                        )
                    ot = sp.tile([128, TQ], f32, name="ot")
                    nc.vector.tensor_copy(out=ot[:], in_=pso[:])
                    nc.sync.dma_start(
                        out=out_d[it * 128 : (it + 1) * 128, oc * TQ : (oc + 1) * TQ],
                        in_=ot[:],
                    )

    nc.finalize()
    return nc


def _bf16(a):
    return np.ascontiguousarray(a, dtype=np.float32).astype(ml_dtypes.bfloat16)


def _core_inputs(x, w_attn, b_attn, w_proj, masks, core):
    b, g = divmod(core, 2)
    gs = slice(g * CL, (g + 1) * CL)
    wq, wk, wv_ = (w_attn[i * C : (i + 1) * C][gs] for i in range(3))
    bq, bk, bv_ = (b_attn[i * C : (i + 1) * C][gs] for i in range(3))

    wqkT = np.concatenate([wq, wk], 0).T            # [C, 2*CL]
    wvT = wv_.T                                     # [C, CL]
    wv_aug = np.zeros((C, VW), np.float32)
    bv_aug = np.zeros((1, VW), np.float32)
    for h in range(HL):
        wv_aug[:, h * 65 : h * 65 + 64] = wvT[:, h * 64 : (h + 1) * 64]
        bv_aug[0, h * 65 : h * 65 + 64] = bv_[h * 64 : (h + 1) * 64]
        bv_aug[0, h * 65 + 64] = 1.0                # softmax denominator column

    return {
        "xt": _bf16(x[b].T).reshape(8, 128, T),
        "wqk": _bf16(wqkT).reshape(8, 128, 2 * CL),
        "wv": _bf16(wv_aug).reshape(8, 128, VW),
        "bqk": _bf16(np.concatenate([bq, bk])[None, :]),
        "bv": _bf16(bv_aug),
        "wp": _bf16(w_proj[:, gs].T).reshape(4, 128, C),
        "mask": masks,
    }


def _make_masks():
    qq = np.arange(TQ)[None, :]
    kk = np.arange(TK)[:, None]
    m = np.stack([(qq >= kk + j * TK) for j in range(4)]).astype(np.float32)
    return m.astype(ml_dtypes.bfloat16)


def kernel(x, w_attn, b_attn, w_proj, b_proj):
    global _prog, last_results
    from concourse.bass_utils import run_bass_kernel_spmd

    if _prog is None:
        _prog = _build_program()

    x = np.asarray(x, np.float32)
    w_attn = np.asarray(w_attn, np.float32)
    b_attn = np.asarray(b_attn, np.float32)
    w_proj = np.asarray(w_proj, np.float32)
    b_proj = np.asarray(b_proj, np.float32)

    masks = _make_masks()
    in_maps = [
        _core_inputs(x, w_attn, b_attn, w_proj, masks, core) for core in range(8)
    ]
    kwargs = {}
    tmpdir = os.environ.get("BASS_TMPDIR")
    if tmpdir:
        os.makedirs(tmpdir, exist_ok=True)
        kwargs["tmpdir"] = tmpdir
    res = run_bass_kernel_spmd(_prog, in_maps, list(range(8)), **kwargs)
    last_results = res

    out = np.empty((B, T, C), np.float32)
    for b in range(B):
        out[b] = res.results[2 * b]["out"] + res.results[2 * b + 1]["out"] + b_proj
    return out



# revision 9
# speedup vs baseline: 1.3146x; 1.3146x over previous
"""Causal self-attention (B=4, T=2048, C=1024, H=16) on 8 NeuronCores.

Sharding: core = (batch b, head-group g): data-parallel over B=4, tensor-
parallel over heads (2 groups x 8 heads).  Each core computes QKV + attention
for its 8 heads and the matching half of the c_proj contraction; the host
sums the two partial c_proj outputs per batch and adds b_proj.

Device layout notes:
  - all matmul operands bf16 (PE runs fp32 at 1/4 rate), PSUM f32
  - x, weights are pre-transposed on the host so every matmul contraction
    sits on the partition dim; no on-device transposes anywhere
  - QKV biases enter as K=1 rank-1 matmuls against a ones row
  - S is computed transposed ([keys, queries]); exp(S/8) on ScalarE with no
    max-subtraction (logits bounded ~+-4 for this problem's scale)
  - causality at tile granularity: k-tiles above the diagonal are skipped,
    diagonal tiles multiplied by precomputed 0/1 masks after exp; diagonal
    tiles further restrict S/exp/PV to their valid column range
  - softmax denominator = ones column appended to each head's V; PV matmul
    emits [y.T | denom] per (head, q-chunk); normalization = batched DVE
    reciprocal + one-hot-selector broadcast matmul + elementwise multiply

Scheduling notes (the performance-critical part):
  - the attention inner loop is ScalarE(exp)-bound; the PE would idle ~35%
    and the HAM activity monitor then clock-throttles it to 1.2 GHz.  To
    keep the PE dense, later head-pairs' QKV projection matmuls (and the
    second half of V) are drip-fed one-per-iteration into the attention
    k-loop as "filler" work, and each head's normalization is deferred into
    the next head's filler stream (its reciprocal would otherwise stall the
    in-order PE queue >3.4us and re-trip the throttle)
  - S+exp run DEPTH=3 k-tiles ahead of the PV consumer (software pipeline)
  - emission order == dependency-tracking order, so ensure() force-emits a
    pair's projection fillers before that pair's attention reads them
"""

import os

import numpy as np
import ml_dtypes

B, T, C, H = 4, 2048, 1024, 16
D = 64          # head dim
HL = 8          # heads per core
CL = HL * D     # 512 local channels
TQ = 512        # query chunk (matmul moving dim)
TK = 128        # key tile (psum partition dim)
NQC = T // TQ   # 4 query chunks
NKT = T // TK   # 16 key tiles
VW = HL * (D + 1)  # 520: V with per-head ones column

_prog = None
last_results = None  # BassKernelResults of the most recent run (for test.py)


def _build_program():
    import concourse.mybir as mybir
    import concourse.tile as tile
    from concourse import bacc

    f32 = mybir.dt.float32
    bf16 = mybir.dt.bfloat16
    EXP = mybir.ActivationFunctionType.Exp

    nc = bacc.Bacc("TRN2", target_bir_lowering=False, debug=False)

    xt_d = nc.dram_tensor("xt", [8, 128, T], bf16, kind="ExternalInput")
    wqk_d = nc.dram_tensor("wqk", [8, 128, 2 * CL], bf16, kind="ExternalInput")
    wv_d = nc.dram_tensor("wv", [8, 128, VW], bf16, kind="ExternalInput")
    # per-partition bias columns: bqkc[:, dst*4+g] = bias for (q|k, pair g);
    # bvb = bv broadcast down 128 partitions (incl. the ones column) so the
    # bias adds ride the psum->sbuf copies instead of K=1 rank-1 matmuls
    bqkc_d = nc.dram_tensor("bqkc", [128, 8], f32, kind="ExternalInput")
    bvb_d = nc.dram_tensor("bvb", [128, VW], f32, kind="ExternalInput")
    wp_d = nc.dram_tensor("wp", [4, 128, C], bf16, kind="ExternalInput")
    mask_d = nc.dram_tensor("mask", [4, 128, TQ], bf16, kind="ExternalInput")
    out_d = nc.dram_tensor("out", [T, C], bf16, kind="ExternalOutput")

    with tile.TileContext(nc) as tc:
        with (
            tc.tile_pool(name="persist", bufs=1) as pp,
            tc.tile_pool(name="ptpool", bufs=8) as ptp,
            tc.tile_pool(name="stage", bufs=3) as sp,
            tc.tile_pool(name="small", bufs=3) as smp,
            tc.tile_pool(name="psA", bufs=4, space="PSUM") as psA,
            tc.tile_pool(name="psF", bufs=2, space="PSUM") as psF,
            tc.tile_pool(name="psY", bufs=1, space="PSUM") as psY,
            tc.tile_pool(name="psB", bufs=1, space="PSUM") as psB,
        ):
            # ---- load everything ----
            xt = [pp.tile([128, T], bf16, name=f"xt{k}") for k in range(8)]
            wqk = [pp.tile([128, 2 * CL], bf16, name=f"wqk{k}") for k in range(8)]
            wv = [pp.tile([128, VW], bf16, name=f"wv{k}") for k in range(8)]
            wp = [pp.tile([128, C], bf16, name=f"wp{k}") for k in range(4)]
            maskt = [pp.tile([128, TQ], bf16, name=f"mask{j}") for j in range(4)]
            bqkc = pp.tile([128, 8], f32, name="bqkc")
            bvb = pp.tile([128, VW], f32, name="bvb")
            # one-hot selector matrices: bcast of row 32*s of a [97,512] tile
            # into 64 partitions via a rank-1 matmul (SBUF APs may only start
            # at partition 0/32/64/96, hence the 32-pitch); bf16 so the bcast
            # matmul takes the fast PE path (fp32 LOW_HIGH is ~4x slower)
            sel = [pp.tile([97, 64], bf16, name=f"sel{i}") for i in range(4)]

            # interleave wqk/xt so QKV matmul k-step j can start as soon as
            # its two tiles land (~2us) instead of after the full input load
            for k in range(8):
                nc.sync.dma_start(out=wqk[k][:], in_=wqk_d[k])
                nc.sync.dma_start(out=xt[k][:], in_=xt_d[k])
                if k == 0:
                    nc.sync.dma_start(out=bqkc[:], in_=bqkc_d[:])
                    nc.sync.dma_start(out=bvb[:], in_=bvb_d[:])
            for k in range(8):
                nc.sync.dma_start(out=wv[k][:], in_=wv_d[k])
            for j in range(4):
                nc.sync.dma_start(out=maskt[j][:], in_=mask_d[j])
            for k in range(4):
                nc.sync.dma_start(out=wp[k][:], in_=wp_d[k])
            for i in range(4):
                nc.vector.memset(sel[i][:], 0.0)
                nc.vector.memset(sel[i][32 * i : 32 * i + 1, :], 1.0)

            # ---- QKV projection ----
            # QT/KT in [channel, t] layout; channel tile g = head pair g
            qt = [pp.tile([128, T], bf16, name=f"qt{i}") for i in range(4)]
            kt = [pp.tile([128, T], bf16, name=f"kt{i}") for i in range(4)]
            # V in natural [t, channel] layout with a ones column per head
            vsb = [pp.tile([128, VW], bf16, name=f"v{i}") for i in range(NKT)]
            yt = [pp.tile([128, T], bf16, name=f"yt{i}") for i in range(4)]

            def emit_qk_steps(g):
                """One head-pair's Q.T and K.T projection as a list of
                single-matmul closures (PE filler units)."""
                steps = []
                for dst, off, bi in ((qt, 0, 0), (kt, CL, 4)):
                    for j in range(NQC):
                        ph = {}

                        def step(k, ph=ph, dst=dst, off=off, j=j, g=g, bi=bi):
                            if k == 0:
                                ph["ps"] = psF.tile(
                                    [128, TQ], f32, name="ps_f", tag="fill"
                                )
                            nc.tensor.matmul(
                                ph["ps"][:],
                                lhsT=wqk[k][:, off + g * 128 : off + (g + 1) * 128],
                                rhs=xt[k][:, j * TQ : (j + 1) * TQ],
                                start=(k == 0),
                                stop=(k == 7),
                            )
                            if k == 7:
                                # bias add rides the psum->sbuf copy (DVE)
                                nc.vector.tensor_scalar_add(
                                    out=dst[g][:, j * TQ : (j + 1) * TQ],
                                    in0=ph["ps"][:],
                                    scalar1=bqkc[:, bi + g : bi + g + 1],
                                )

                        for k in range(8):
                            steps.append(lambda k=k, step=step: step(k))
                return steps

            def emit_v_steps(h2):
                """V projection for 4 heads as single-matmul closures."""
                w0 = h2 * (VW // 2)
                steps = []
                for it in range(NKT):
                    ph = {}

                    def step(k, ph=ph, it=it, w0=w0):
                        if k == 0:
                            ph["ps"] = psF.tile([128, TQ], f32, name="ps_v",
                                                tag="fill")
                        nc.tensor.matmul(
                            ph["ps"][:, : VW // 2],
                            lhsT=xt[k][:, it * 128 : (it + 1) * 128],
                            rhs=wv[k][:, w0 : w0 + VW // 2],
                            start=(k == 0),
                            stop=(k == 7),
                        )
                        if k == 7:
                            # bias (incl. the denominator ones column) rides
                            # the psum->sbuf copy via the broadcast tile
                            nc.vector.tensor_add(
                                out=vsb[it][:, w0 : w0 + VW // 2],
                                in0=ph["ps"][:, : VW // 2],
                                in1=bvb[:, w0 : w0 + VW // 2],
                            )

                    for k in range(8):
                        steps.append(lambda k=k, step=step: step(k))
                return steps

            # pre-phase (dense PE work, warms HAM): pair 0 QK + first V half
            # (heads 0-3); the second half drains via the filler stream well
            # before attn pair 2 (heads 4-7) consumes it
            for step in emit_qk_steps(0):
                step()
            for step in emit_v_steps(0):
                step()

            # ---- attention ----
            # the attention inner loop is ScalarE(exp)-bound; drip-feed the
            # NEXT pair's QKV matmuls into the PE queue as filler so the PE
            # stays dense (otherwise HAM clock-throttles it to 1.2 GHz)
            fillers = []  # list of (tag, closure)

            def drain(n):
                for _ in range(n):
                    if fillers:
                        fillers.pop(0)[1]()

            def ensure(tag):
                """Force-emit every queued step up to the last one of `tag`
                (emission order == dependency-tracking order, so a pair's
                projection steps MUST be emitted before its attention reads)."""
                while any(t == tag for t, _ in fillers):
                    fillers.pop(0)[1]()

            for h in range(HL):
                g2, po = h // 2, 64 * (h % 2)
                if h % 2 == 0 and g2 < 3:
                    fillers.extend((f"qk{g2 + 1}", s) for s in emit_qk_steps(g2 + 1))
                    if g2 == 0:
                        fillers.extend(("vh1", s) for s in emit_v_steps(1))
                if h % 2 == 0:
                    ensure(f"qk{g2}")
                    if g2 >= 2:
                        ensure("vh1")
                den_g = smp.tile([97, TQ], f32, name="den_g", tag="deng", bufs=4)
                nc.vector.memset(den_g[:], 1.0)  # rows between dens unused
                slots = []
                for qc in range(NQC):
                    ktop = (qc + 1) * (TQ // TK)  # causal: k tiles 0..ktop-1
                    yps = psY.tile([D + 1, TQ], f32, name="yps", tag="y")

                    # software pipeline: S+exp runs DEPTH tiles ahead of the
                    # PV consumer so the PE never waits on the exp
                    DEPTH = 3
                    pts = {}

                    def s_stage(ktl, qc=qc, g2=g2, po=po, pts=pts):
                        j = ktl - qc * (TQ // TK)
                        # diagonal tiles have no valid columns before col0
                        col0 = j * TK if j >= 0 else 0
                        ps_s = psA.tile([128, TQ], f32, name="ps_s", tag="mm512")
                        nc.tensor.matmul(
                            ps_s[:, col0:],
                            lhsT=kt[g2][po : po + 64, ktl * TK : (ktl + 1) * TK],
                            rhs=qt[g2][po : po + 64, qc * TQ + col0 : (qc + 1) * TQ],
                            start=True,
                            stop=True,
                        )
                        pt_t = ptp.tile([128, TQ], bf16, name="pt")
                        # P.T = exp(S.T/sqrt(D)); logits bounded, no max pass
                        nc.scalar.activation(
                            pt_t[:, col0:], ps_s[:, col0:], EXP, scale=0.125
                        )
                        if j >= 0:  # diagonal: zero the acausal corner
                            # on GPSIMD (idle): keeps the mask off the busy
                            # in-order DVE queue so PV never waits behind
                            # ysb/den/filler copies
                            nc.gpsimd.tensor_mul(
                                pt_t[:, col0:], pt_t[:, col0:], maskt[j][:, col0:]
                            )
                        pts[ktl] = (pt_t, col0)

                    def pv_stage(ktl, qc=qc, h=h, pts=pts, yps=yps, ktop=ktop):
                        pt_t, col0 = pts.pop(ktl)
                        nc.tensor.matmul(
                            yps[:, col0:],
                            lhsT=vsb[ktl][:, h * 65 : (h + 1) * 65],
                            rhs=pt_t[:, col0:],
                            start=(ktl == 0),
                            stop=(ktl == ktop - 1),
                        )

                    for ktl in range(ktop):
                        s_stage(ktl)
                        drain(2 if len(fillers) >= 80 else 1)
                        if ktl >= DEPTH:
                            pv_stage(ktl - DEPTH)
                    for ktl in range(max(0, ktop - DEPTH), ktop):
                        pv_stage(ktl)
                    # rows 0..63 = unnormalized y.T, row 64 = denominator
                    ysb = smp.tile([D + 1, TQ], f32, name="ysb", tag="ysb", bufs=14)
                    nc.vector.tensor_copy(out=ysb[:], in_=yps[:])
                    nc.vector.tensor_copy(
                        out=den_g[32 * qc : 32 * qc + 1, :], in_=ysb[64:65, :]
                    )
                    slots.append((h, qc, ysb))
                # batched 1/den for the head (DVE; free-size bound so one
                # [97,512] reciprocal costs the same as [1,512]).  The whole
                # normalization is pushed into the NEXT head's filler stream:
                # emitted inline it stalls the in-order PE queue ~4us at every
                # head boundary (reciprocal latency), re-tripping the HAM
                # throttle.  approx_fast (~51 ULP) is ~5x cheaper than the
                # exact reciprocal; rec is cast to bf16 so the bcast matmul
                # takes the fast PE path.
                rec_f = smp.tile([97, TQ], f32, name="rec_f", tag="recf", bufs=2)
                rec_g = smp.tile([97, TQ], bf16, name="rec_g", tag="recg", bufs=3)

                def norm_steps(rec_f=rec_f, rec_g=rec_g, den_g=den_g,
                               slots=tuple(slots)):
                    steps = [
                        lambda: nc.vector.reciprocal_approx_fast(
                            out=rec_f[:], in_=den_g[:]
                        ),
                        lambda: nc.vector.tensor_copy(out=rec_g[:], in_=rec_f[:]),
                    ]

                    def one(h_, qc, ysb):
                        g2_, po_ = h_ // 2, 64 * (h_ % 2)
                        bc = psB.tile([64, TQ], f32, name="bc", tag="b")
                        nc.tensor.matmul(
                            bc[:],
                            lhsT=sel[qc][:],
                            rhs=rec_g[:],
                            start=True,
                            stop=True,
                        )
                        nc.vector.tensor_mul(
                            yt[g2_][po_ : po_ + 64, qc * TQ : (qc + 1) * TQ],
                            ysb[0:64, :],
                            bc[:],
                        )

                    for h_, qc, ysb in slots:
                        steps.append(lambda h_=h_, qc=qc, ysb=ysb: one(h_, qc, ysb))
                    return steps

                # a few slots of lead so the reciprocal completes before its
                # broadcast matmuls reach the PE, but early enough that ysb
                # tiles are recycled within ~one head
                for i_, st in enumerate(norm_steps()):
                    fillers.insert(min(8 + i_, len(fillers)), ("norm", st))
            drain(len(fillers))

            # ---- c_proj (local half of the contraction) ----
            for it in range(NKT):
                for oc in range(2):
                    pso = psA.tile([128, TQ], f32, name="ps_o", tag="mm512")
                    for ic in range(4):
                        nc.tensor.matmul(
                            pso[:],
                            lhsT=yt[ic][:, it * 128 : (it + 1) * 128],
                            rhs=wp[ic][:, oc * TQ : (oc + 1) * TQ],
                            start=(ic == 0),
                            stop=(ic == 3),
                        )
                    ot = sp.tile([128, TQ], bf16, name="ot")
                    nc.vector.tensor_copy(out=ot[:], in_=pso[:])
                    nc.sync.dma_start(
                        out=out_d[it * 128 : (it + 1) * 128, oc * TQ : (oc + 1) * TQ],
                        in_=ot[:],
                    )

    nc.finalize()
    return nc


def _bf16(a):
    return np.ascontiguousarray(a, dtype=np.float32).astype(ml_dtypes.bfloat16)


def _core_inputs(x, w_attn, b_attn, w_proj, masks, core):
    b, g = divmod(core, 2)
    gs = slice(g * CL, (g + 1) * CL)
    wq, wk, wv_ = (w_attn[i * C : (i + 1) * C][gs] for i in range(3))
    bq, bk, bv_ = (b_attn[i * C : (i + 1) * C][gs] for i in range(3))

    wqkT = np.concatenate([wq, wk], 0).T            # [C, 2*CL]
    wvT = wv_.T                                     # [C, CL]
    wv_aug = np.zeros((C, VW), np.float32)
    bv_aug = np.zeros((1, VW), np.float32)
    for h in range(HL):
        wv_aug[:, h * 65 : h * 65 + 64] = wvT[:, h * 64 : (h + 1) * 64]
        bv_aug[0, h * 65 : h * 65 + 64] = bv_[h * 64 : (h + 1) * 64]
        bv_aug[0, h * 65 + 64] = 1.0                # softmax denominator column

    bqk_cat = np.concatenate([bq, bk])              # [2*CL]
    bqkc = np.stack(
        [bqk_cat[j * 128 : (j + 1) * 128] for j in range(8)], axis=1
    ).astype(np.float32)                            # [128, 8]

    return {
        "xt": _bf16(x[b].T).reshape(8, 128, T),
        "wqk": _bf16(wqkT).reshape(8, 128, 2 * CL),
        "wv": _bf16(wv_aug).reshape(8, 128, VW),
        "bqkc": bqkc,
        "bvb": np.ascontiguousarray(
            np.broadcast_to(bv_aug, (128, VW)), np.float32
        ),
        "wp": _bf16(w_proj[:, gs].T).reshape(4, 128, C),
        "mask": masks,
    }


def _make_masks():
    qq = np.arange(TQ)[None, :]
    kk = np.arange(TK)[:, None]
    m = np.stack([(qq >= kk + j * TK) for j in range(4)]).astype(np.float32)
    return m.astype(ml_dtypes.bfloat16)


def kernel(x, w_attn, b_attn, w_proj, b_proj):
    global _prog, last_results
    from concourse.bass_utils import run_bass_kernel_spmd

    if _prog is None:
        _prog = _build_program()

    x = np.asarray(x, np.float32)
    w_attn = np.asarray(w_attn, np.float32)
    b_attn = np.asarray(b_attn, np.float32)
    w_proj = np.asarray(w_proj, np.float32)
    b_proj = np.asarray(b_proj, np.float32)

    masks = _make_masks()
    in_maps = [
        _core_inputs(x, w_attn, b_attn, w_proj, masks, core) for core in range(8)
    ]
    kwargs = {}
    tmpdir = os.environ.get("BASS_TMPDIR")
    if tmpdir:
        os.makedirs(tmpdir, exist_ok=True)
        kwargs["tmpdir"] = tmpdir
    res = run_bass_kernel_spmd(_prog, in_maps, list(range(8)), **kwargs)
    last_results = res

    out = np.empty((B, T, C), np.float32)
    for b in range(B):
        out[b] = (
            np.asarray(res.results[2 * b]["out"], np.float32)
            + np.asarray(res.results[2 * b + 1]["out"], np.float32)
            + b_proj
        )
    return out

